# revision 83
# baseline (speedup 1.0000x reference)
"""Distributed Bass kernel for nn_Attention (B=2, S=2048, HID=2048, H=32, KVH=8, D=64).

Sharding (8 cores, uniform SPMD): core c owns kv-head c and its 4 GQA query
heads (2 pairs).  x replicated as xT [HID, T] bf16.

Pipeline (emitted interleaved so all engines overlap):
  per 512-token chunk t (b = t//4, cq = t%4):
    - one 2MB DMA for the x chunk, one DMA per trig table chunk
    - QKV projections (PE) + RoPE (DVE) -> qT/k2/vatt
    - causal attention for (b, cq), both head pairs: per 128-key block one
      psS [128, 2*512] holds both heads' scores (S^T matmuls on disjoint PE
      row groups run concurrently); one EXP (ACT) covers both; AV shares one
      LDWEIGHTS.  Denominator comes from a ones-column in vatt (psO row 64).
  - after every 2 chunks (1024 tokens), an AllToAll redistributes that token
    group's unnormalized attn^T (+2 denominator rows in-band) so every core
    gets its 128-token slice; normalization (reciprocal + broadcast + one
    wide multiply) and the wo projection for those tokens run overlapped
    with the remaining attention.
Output: core c writes out rows [128u + r] = flat token 1024u + 128c + r.
"""

import numpy as np
import ml_dtypes

import concourse.bass as bass
import concourse.mybir as mybir
import concourse.tile as tile
from concourse import bacc
from concourse.bass_utils import run_bass_kernel_spmd

BF16 = ml_dtypes.bfloat16
F32 = np.float32

B, S, HID = 2, 2048, 2048
H, KVH, D = 32, 8, 64
NC = 8
T = B * S              # 4096 flat tokens
LH = H // NC           # 4 local q-heads (2 pairs)
TC = 512               # token chunk
NTC = T // TC          # 8 chunks
KB = 128               # key block
NG = 4                 # a2a token groups (1024 flat tokens each)
GT = T // NG // NC     # 128 tokens per core per group

_CACHE = {}


def _build():
    import os
    DBG = os.environ.get("KDEBUG", "0") == "1"
    fp32 = mybir.dt.float32
    bf16 = mybir.dt.bfloat16

    nc = bacc.Bacc("TRN2", target_bir_lowering=False, debug=False, num_devices=NC)

    # all inputs pre-tiled on host so every DMA is one contiguous run per
    # partition (128 descriptors instead of 2048)
    xT = nc.dram_tensor("xT", [128, NTC, 16, TC], bf16, kind="ExternalInput")
    wq_c = nc.dram_tensor("wq_c", [128, 16, LH * D], bf16, kind="ExternalInput")
    wkv_c = nc.dram_tensor("wkv_c", [128, 16, 2 * D], bf16,
                           kind="ExternalInput")
    wo_d = nc.dram_tensor("wo", [128, 4, 16, 512], bf16, kind="ExternalInput")
    trigq_d = nc.dram_tensor("trigq", [128, NTC, 2, TC], bf16,
                             kind="ExternalInput")
    trigk_d = nc.dram_tensor("trigk", [64, NTC, 2, TC], bf16,
                             kind="ExternalInput")
    maD_d = [nc.dram_tensor(f"maD{j}", [128, 2 * TC], bf16, kind="ExternalInput")
             for j in range(4)]
    out_d = nc.dram_tensor("out", [NG * GT, HID], fp32, kind="ExternalOutput")
    if DBG:
        dbg_qT = nc.dram_tensor("dbg_qT", [2, 128, T], fp32, kind="ExternalOutput")
        dbg_k2 = nc.dram_tensor("dbg_k2", [128, T], fp32, kind="ExternalOutput")
        dbg_attnT = nc.dram_tensor("dbg_attnT", [2, 128, T], fp32,
                                   kind="ExternalOutput")
        dbg_den = nc.dram_tensor("dbg_den", [NG, 2, 2, 1024], fp32,
                                 kind="ExternalOutput")
        dbg_ao = nc.dram_tensor("dbg_ao", [NG, 128, 16, GT], fp32,
                                kind="ExternalOutput")
        dbg_rcpb = nc.dram_tensor("dbg_rcpb", [NG, 128, 16, GT], fp32,
                                  kind="ExternalOutput")
        dbg_denall = nc.dram_tensor("dbg_denall", [NG, 64, GT], fp32,
                                    kind="ExternalOutput")
        dbg_rcp = nc.dram_tensor("dbg_rcp", [NG, 64, GT], fp32,
                                 kind="ExternalOutput")


    with tile.TileContext(nc) as tc:
        with (
            tc.tile_pool(name="persist", bufs=1) as persist,
            tc.tile_pool(name="stream", bufs=2) as stream,
            tc.tile_pool(name="work", bufs=2) as work,
            tc.tile_pool(name="psum", bufs=1, space="PSUM") as psum,
            tc.tile_pool(name="dram", bufs=1, space="DRAM") as dram,
        ):
            # ---- prologue: exp table load warm-up ----
            dummy = work.tile([1, 2], fp32, tag="dummy", name="dummy")
            nc.gpsimd.memset(dummy[:], 0.0)
            nc.scalar.activation(dummy[:], dummy[:],
                                 mybir.ActivationFunctionType.Exp)

            # ---- persistent tiles ----
            qT = [persist.tile([128, T], bf16, tag=f"qT{t}", name=f"qT{t}")
                  for t in range(2)]
            k2 = persist.tile([128, T], bf16, tag="k2", name="k2")
            vatt = [persist.tile([128, D + 1], bf16, tag=f"vatt{i}",
                                 name=f"vatt{i}") for i in range(T // KB)]
            attnT = [persist.tile([128, T], bf16, tag=f"attnT{t}",
                                  name=f"attnT{t}") for t in range(2)]
            den_tiles = {}  # (group u, pair p) -> [1, 2, 1024] bf16 tile
            ident = persist.tile([128, 128], bf16, tag="ident", name="ident")

            # weights: single big DMAs on the ACT HWDGE ring (scalar queue is
            # otherwise idle until the first EXP), so they overlap the x
            # chunk-0 load on the SP ring
            wq_sb = persist.tile([128, 16, LH * D], bf16, tag="wq", name="wq")
            wkv_sb = persist.tile([128, 16, 2 * D], bf16, tag="wkv", name="wkv")
            nc.scalar.dma_start(wq_sb[:], wq_c[:])
            nc.scalar.dma_start(wkv_sb[:], wkv_c[:])

            maD = []
            for j in range(4):
                mt = persist.tile([128, 2 * TC], bf16, tag=f"maD{j}",
                                  name=f"maD{j}")
                nc.gpsimd.dma_start(mt[:], maD_d[j][:])
                maD.append(mt)
            from concourse.masks import make_identity
            make_identity(nc, ident[:])
            for i in range(T // KB):
                nc.gpsimd.memset(vatt[i][:, D:D + 1], 1.0)

            # wo: 4 column blocks, resident
            wo_sb = []
            for nt in range(4):
                t_ = persist.tile([128, 16, 512], bf16, tag=f"wo{nt}",
                                  name=f"wo{nt}")
                wo_sb.append(t_)

            # a2a staging (one group = 1024 flat tokens; slot j -> core j's
            # 128 tokens; rows 0:128 attnT block, 128:130 the 2 denominators).
            # One collective per (group, pair) so pair 0's redistribution
            # overlaps pair 1's attention.
            a2a_in = [[dram.tile([NC, 130, GT], bf16, tag=f"a2a_in{u}_{p}",
                                 name=f"a2a_in{u}_{p}") for p in range(2)]
                      for u in range(NG)]
            a2a_out = [[dram.tile([NC, 130, GT], bf16, tag=f"a2a_out{u}_{p}",
                                  name=f"a2a_out{u}_{p}") for p in range(2)]
                       for u in range(NG)]
            rstage = [dram.tile([2, 2, 8, GT], bf16, tag=f"rstage{u}",
                                name=f"rstage{u}") for u in range(NG)]

            def rope_q(out_ap, ps, tg, tsl):
                ct = tg[:, 0, :]
                st = tg[:, 1, :]
                t1 = work.tile([128, TC], fp32, tag="rope_t1", bufs=1, name="t1")
                t2 = work.tile([128, TC], fp32, tag="rope_t2", bufs=1, name="t2")
                nc.vector.tensor_mul(t1[:], ps[:], ct)
                for base in range(0, 128, 64):
                    a, b = base, base + 32
                    nc.vector.tensor_mul(t2[a:a + 32, :], ps[b:b + 32, :],
                                         st[a:a + 32, :])
                    nc.vector.tensor_mul(t2[b:b + 32, :], ps[a:a + 32, :],
                                         st[b:b + 32, :])
                nc.vector.tensor_add(out_ap, t1[:], t2[:])

            def rope_k(out_ap, ps, tg):
                ct = tg[:, 0, :]
                st = tg[:, 1, :]
                t1 = work.tile([64, TC], fp32, tag="rope_t1", bufs=1, name="kt1")
                t2 = work.tile([64, TC], fp32, tag="rope_t2", bufs=1, name="kt2")
                nc.vector.tensor_mul(t1[:], ps[0:64, :], ct)
                nc.vector.tensor_mul(t2[0:32, :], ps[32:64, :], st[0:32, :])
                nc.vector.tensor_mul(t2[32:64, :], ps[0:32, :], st[32:64, :])
                nc.vector.tensor_add(out_ap, t1[:], t2[:])

            def emit_a2a(u, p):
                """Stage + send pair p's slice of group u."""
                nc.gpsimd.dma_start(
                    a2a_in[u][p][:, 0:128, :].rearrange("j q t -> q j t"),
                    attnT[p][:, 1024 * u:1024 * (u + 1)]
                    .rearrange("q (j t) -> q j t", j=NC))
                for h in range(2):
                    nc.gpsimd.dma_start(
                        a2a_in[u][p][:, 128 + h, :],
                        den_tiles[(u, p)][:, 1024 * h:1024 * (h + 1)]
                        .rearrange("o (j t) -> o j t", j=NC))
                nc.gpsimd.collective_compute(
                    "AllToAll", mybir.AluOpType.bypass,
                    replica_groups=[list(range(NC))],
                    ins=[a2a_in[u][p].opt()], outs=[a2a_out[u][p].opt()])
                if DBG:
                    nc.gpsimd.dma_start(
                        dbg_den[u, p, :, :],
                        den_tiles[(u, p)][:, :]
                        .rearrange("o (h t) -> o h t", h=2))

            p2_state = {}

            def _setup_parity(u, pp):
                """Load + normalize parity-pp slots (kk = 2r + pp) of group u."""
                if u not in p2_state:
                    p2_state[u] = (
                        stream.tile([128, 16, GT], bf16, tag="aoall", bufs=1,
                                    name=f"aoall{u}"),
                        work.tile([64, GT], bf16, tag="den_all",
                                  name=f"den_all{u}"),
                        work.tile([64, GT], bf16, tag="rcp", name=f"rcp{u}"),
                        stream.tile([128, 16, GT], bf16, tag="rcpb", bufs=1,
                                    name=f"rcpb{u}"))
                aoall, den_all, rcp, rcpb = p2_state[u]
                for r in range(NC):
                    eng = nc.sync if r % 2 == 0 else nc.gpsimd
                    eng.dma_start(aoall[:, 2 * r + pp, :],
                                  a2a_out[u][pp][r, 0:128, :])
                # den_all rows: 32*pp + 8*h + r (parity halves 32-aligned for
                # the DVE reciprocal)
                for h in range(2):
                    nc.sync.dma_start(
                        den_all[32 * pp + 8 * h:32 * pp + 8 * h + 8, :],
                        a2a_out[u][pp][:, 128 + h, :])
                with nc.allow_low_precision(reason="bf16 softmax denominators"):
                    nc.vector.reciprocal(rcp[32 * pp:32 * pp + 16, :],
                                         den_all[32 * pp:32 * pp + 16, :])
                nc.sync.dma_start(
                    rstage[u][pp].rearrange("h r t -> (h r) t"),
                    rcp[32 * pp:32 * pp + 16, :])
                rcpb4 = rcpb[:].rearrange("q (r pp) t -> q r pp t", pp=2)
                for h in range(2):
                    nc.gpsimd.dma_start(
                        rcpb4[64 * h:64 * (h + 1), :, pp, :],
                        rstage[u][pp:pp + 1, h, :, :].broadcast_to([64, 8, GT]))
                ao4 = aoall[:].rearrange("q (r pp) t -> q r pp t", pp=2)
                nc.vector.tensor_mul(ao4[:, :, pp, :], ao4[:, :, pp, :],
                                     rcpb4[:, :, pp, :])
                if DBG and pp == 1:
                    nc.gpsimd.dma_start(dbg_rcpb[u, :, :, :], rcpb[:])
                    nc.gpsimd.dma_start(dbg_denall[u, :, :], den_all[:])
                    nc.gpsimd.dma_start(dbg_rcp[u, :, :], rcp[:])
                    nc.gpsimd.dma_start(dbg_ao[u, :, :, :], aoall[:])

            def _p2_mm(u, nt, ps_ap, kks):
                aoall = p2_state[u][0]
                for kk in kks:
                    nc.tensor.matmul(ps_ap, aoall[:, kk, :],
                                     wo_sb[nt][:, kk, :],
                                     start=(kk == 0), stop=(kk == 15))

            def _p2_drain(u, nt, ps_ap):
                ob = work.tile([128, 512], fp32, tag="ob", name=f"ob{u}_{nt}")
                nc.vector.tensor_copy(ob[:], ps_ap)
                nc.gpsimd.dma_start(
                    out_d[GT * u:GT * (u + 1), 512 * nt:512 * (nt + 1)], ob[:])

            def _p3_accums():
                tiles = p2_state["ps3"]
                return [tiles[0][:], tiles[1][:],
                        tiles[2][:, 0:512], tiles[2][:, 512:1024]]

            def emit_phase2(u, half):
                """Consume group u: normalize + project its 128 token rows."""
                if u < 3:
                    if half == 0:
                        _setup_parity(u, 0)
                        _setup_parity(u, 1)
                    for nt in (0, 1) if half == 0 else (2, 3):
                        ps = psum.tile([128, 512], fp32, tag="mm", bufs=2,
                                       name=f"ps_o{u}_{nt}")
                        _p2_mm(u, nt, ps[:], range(16))
                        _p2_drain(u, nt, ps[:])
                    return
                # u == 3 (tail): evens (pair-0 data) first across all 4 nt so
                # the PE starts before pair 1's collective lands.
                if half == 0:
                    _setup_parity(u, 0)
                    p2_state["ps3"] = [
                        psum.tile([128, 512], fp32, tag="mm", bufs=2,
                                  name="ps_o3_0"),
                        psum.tile([128, 512], fp32, tag="mm", bufs=2,
                                  name="ps_o3_1"),
                        psum.tile([128, 1024], fp32, tag="psO", bufs=1,
                                  name="ps_o3_23")]
                    for nt, ps_ap in enumerate(_p3_accums()):
                        _p2_mm(u, nt, ps_ap, range(0, 16, 2))
                else:
                    _setup_parity(u, 1)
                    for nt, ps_ap in enumerate(_p3_accums()):
                        _p2_mm(u, nt, ps_ap, range(1, 16, 2))
                        _p2_drain(u, nt, ps_ap)

            # ================= main interleaved loop =================
            for t8 in range(NTC):
                b, cq = t8 // 4, t8 % 4
                tsl = slice(TC * t8, TC * (t8 + 1))

                xq = stream.tile([128, 16, TC], bf16, tag="xq", name=f"xq{t8}")
                if t8 == 0:
                    # split so the first QKV matmuls start on partial data
                    for kq in range(0, 16, 4):
                        nc.sync.dma_start(xq[:, kq:kq + 4, :],
                                          xT[:, t8, kq:kq + 4, :])
                else:
                    nc.sync.dma_start(xq[:], xT[:, t8, :, :])
                tgq = stream.tile([128, 2, TC], bf16, tag="tgq", bufs=1,
                                  name=f"tgq{t8}")
                tgk = stream.tile([64, 2, TC], bf16, tag="tgk", bufs=1,
                                  name=f"tgk{t8}")
                nc.sync.dma_start(tgq[:], trigq_d[:, t8, :, :])
                nc.sync.dma_start(tgk[:], trigk_d[:, t8, :, :])
                if t8 < 4:
                    nc.sync.dma_start(wo_sb[t8][:], wo_d[:, t8, :, :])

                # ---- QKV projections + RoPE ----
                for qt in range(2):
                    ps = psum.tile([128, TC], fp32, tag="mm", bufs=2,
                                   name=f"ps_q{t8}_{qt}")
                    for k in range(16):
                        nc.tensor.matmul(ps[:],
                                         wq_sb[:, k, 128 * qt:128 * (qt + 1)],
                                         xq[:, k, :],
                                         start=(k == 0), stop=(k == 15))
                    rope_q(qT[qt][:, tsl], ps, tgq, tsl)

                ps = psum.tile([128, TC], fp32, tag="mm", bufs=2,
                               name=f"ps_kv{t8}")
                for k in range(16):
                    nc.tensor.matmul(ps[:], wkv_sb[:, k, :], xq[:, k, :],
                                     start=(k == 0), stop=(k == 15))
                rope_k(k2[0:64, tsl], ps, tgk)
                nc.vector.tensor_copy(k2[64:128, tsl], k2[0:64, tsl])
                vt = work.tile([64, TC], bf16, tag="vt", name=f"vt{t8}")
                nc.vector.tensor_copy(vt[:], ps[64:128, :])
                for j in range(TC // KB):
                    kbi = (TC // KB) * t8 + j
                    pst = psum.tile([128, TC], bf16, tag="mm", bufs=2,
                                    name=f"ps_tr{t8}_{j}")
                    nc.tensor.transpose(pst[:, 0:64],
                                        vt[:, 128 * j:128 * (j + 1)],
                                        ident[0:64, 0:64])
                    nc.scalar.copy(vatt[kbi][:, 0:D], pst[:, 0:64])

                # ---- attention for (b, cq), both pairs ----
                nkb = 4 * (cq + 1)
                qs = S * b + TC * cq
                gu, goff = qs // 1024, qs % 1024
                for p in range(2):
                    if (gu, p) not in den_tiles:
                        den_tiles[(gu, p)] = work.tile(
                            [1, 2048], bf16, tag=f"deng{p}", bufs=2,
                            name=f"deng{gu}_{p}")
                    qtile = qT[p]
                    psO = psum.tile([D + 1, 1024], fp32, tag="psO", bufs=1,
                                    name=f"psO{t8}_{p}")

                    def emit_av(kb, ex):
                        vt_ = vatt[(S // KB) * b + kb][:]
                        nc.tensor.matmul(psO[:, 0:512], vt_, ex[:, 0:512],
                                         start=(kb == 0), stop=(kb == nkb - 1))
                        nc.tensor.matmul(psO[:, 512:1024], vt_,
                                         ex[:, 512:1024],
                                         start=(kb == 0), stop=(kb == nkb - 1))

                    # 2-kblock skew: S^T(kb) runs while AV(kb-2) waits on
                    # exp/psO, so the in-order PE queue never starves.
                    pend = []
                    for kb in range(nkb):
                        kpos = S * b + KB * kb
                        psS = psum.tile([128, 1024], fp32, tag="psS", bufs=2,
                                        name=f"psS{t8}_{p}_{kb}")
                        nc.tensor.matmul(psS[:, 0:512],
                                         k2[0:64, kpos:kpos + KB],
                                         qtile[0:64, qs:qs + TC],
                                         start=True, stop=True)
                        nc.tensor.matmul(psS[:, 512:1024],
                                         k2[64:128, kpos:kpos + KB],
                                         qtile[64:128, qs:qs + TC],
                                         start=True, stop=True)
                        ex = work.tile([128, 1024], bf16, tag="ex", bufs=3,
                                       name=f"ex{t8}_{p}_{kb}")
                        nc.scalar.activation(ex[:], psS[:],
                                             mybir.ActivationFunctionType.Exp)
                        if kb >= nkb - 4:
                            nc.vector.tensor_mul(ex[:], ex[:],
                                                 maD[kb - (nkb - 4)][:])
                        pend.append((kb, ex))
                        if len(pend) > 2:
                            emit_av(*pend.pop(0))
                    for it in pend:
                        emit_av(*it)
                    # drain
                    nc.vector.tensor_copy(attnT[p][0:64, qs:qs + TC],
                                          psO[0:64, 0:512])
                    nc.vector.tensor_copy(attnT[p][64:128, qs:qs + TC],
                                          psO[0:64, 512:1024])
                    for h in range(2):
                        nc.vector.tensor_copy(
                            den_tiles[(gu, p)][:, 1024 * h + goff:
                                               1024 * h + goff + TC],
                            psO[64:65, 512 * h:512 * (h + 1)])
                    if t8 % 2 == 1:
                        emit_a2a(t8 // 2, p)
                    if t8 >= 3 and t8 % 2 == 1:
                        emit_phase2((t8 - 3) // 2, p)
            emit_phase2(3, 0)
            emit_phase2(3, 1)
            if DBG:
                for p in range(2):
                    nc.gpsimd.dma_start(dbg_qT[p, :, :], qT[p][:])
                    nc.gpsimd.dma_start(dbg_attnT[p, :, :], attnT[p][:])
                nc.gpsimd.dma_start(dbg_k2[:, :], k2[:])

    nc.compile()
    return nc


def _prep_inputs(x, cos, sin, wq, wk, wv, wo):
    x = np.asarray(x, F32)
    cos = np.asarray(cos, F32)
    sin = np.asarray(sin, F32)
    wq = np.asarray(wq, F32)
    wk = np.asarray(wk, F32)
    wv = np.asarray(wv, F32)
    wo = np.asarray(wo, F32)

    # [128, chunk, k, t] = x[512*chunk + t, 128*k + p]
    xT = np.ascontiguousarray(
        x.reshape(NTC, TC, 16, 128).transpose(3, 0, 2, 1)).astype(BF16)
    # [128, nt, k, m] = wo[128*k + p, 512*nt + m]
    wo_b = np.ascontiguousarray(
        wo.reshape(16, 128, 4, 512).transpose(1, 2, 0, 3)).astype(BF16)

    pos = np.arange(T) % S
    sign = np.concatenate([-np.ones(D // 2, F32), np.ones(D // 2, F32)])
    ctk = np.ascontiguousarray(cos[pos].T)                      # [64, T]
    stk = np.ascontiguousarray((sin[pos] * sign).T)             # [64, T]
    sc = F32(1.0 / np.sqrt(D))
    ctq = np.concatenate([ctk, ctk], 0) * sc                    # [128, T]
    stq = np.concatenate([stk, stk], 0) * sc

    def chunked(ct, st):  # [P, T] x2 -> [P, NTC, 2, TC]
        P = ct.shape[0]
        s = np.stack([ct, st], axis=1)          # [P, 2, T]
        return np.ascontiguousarray(
            s.reshape(P, 2, NTC, TC).transpose(0, 2, 1, 3)).astype(BF16)

    trigq = chunked(ctq, stq)
    trigk = chunked(ctk, stk)

    ql = np.arange(TC)
    kl = np.arange(128)
    maD = []
    for j in range(4):
        m = (ql[None, :] >= (kl[:, None] + 128 * j)).astype(BF16)
        maD.append(np.concatenate([m, m], axis=1))              # [128, 1024]

    in_maps = []
    for c in range(NC):
        wq_cc = np.ascontiguousarray(
            wq[:, c * LH * D:(c + 1) * LH * D]
            .reshape(16, 128, LH * D).transpose(1, 0, 2)).astype(BF16)
        wkv_cc = np.ascontiguousarray(np.concatenate(
            [wk[:, c * D:(c + 1) * D], wv[:, c * D:(c + 1) * D]], 1)
            .reshape(16, 128, 2 * D).transpose(1, 0, 2)).astype(BF16)
        in_maps.append({
            "xT": xT, "wq_c": wq_cc, "wkv_c": wkv_cc, "wo": wo_b,
            "trigq": trigq, "trigk": trigk,
            "maD0": maD[0], "maD1": maD[1], "maD2": maD[2], "maD3": maD[3],
        })
    return in_maps


def get_nc():
    if "nc" not in _CACHE:
        _CACHE["nc"] = _build()
    return _CACHE["nc"]


def run(in_maps, **kwargs):
    nc = get_nc()
    return run_bass_kernel_spmd(nc, in_maps, core_ids=list(range(NC)), **kwargs)


def kernel(x, cos, sin, wq, wk, wv, wo):
    in_maps = _prep_inputs(x, cos, sin, wq, wk, wv, wo)
    res = run(in_maps)
    out = np.empty((T, HID), F32)
    for c in range(NC):
        r = res.results[c]["out"]
        for u in range(NG):
            out[1024 * u + GT * c:1024 * u + GT * (c + 1)] = \
                r[GT * u:GT * (u + 1)]
    return out.reshape(B, S, HID)


# revision 84
# speedup vs baseline: 1.0162x; 1.0162x over previous
"""Distributed Bass kernel for nn_Attention (B=2, S=2048, HID=2048, H=32, KVH=8, D=64).

Sharding (8 cores, uniform SPMD): core c owns kv-head c and its 4 GQA query
heads (2 pairs).  x replicated as xT [HID, T] bf16.

Pipeline (emitted interleaved so all engines overlap):
  per 512-token chunk t (b = t//4, cq = t%4):
    - one 2MB DMA for the x chunk, one DMA per trig table chunk
    - QKV projections (PE) + RoPE (DVE) -> qT/k2/vatt
    - causal attention for (b, cq), both head pairs: per 128-key block one
      psS [128, 2*512] holds both heads' scores (S^T matmuls on disjoint PE
      row groups run concurrently); one EXP (ACT) covers both; AV shares one
      LDWEIGHTS.  Denominator comes from a ones-column in vatt (psO row 64).
  - after every 2 chunks (1024 tokens), an AllToAll redistributes that token
    group's unnormalized attn^T (+2 denominator rows in-band) so every core
    gets its 128-token slice; normalization (reciprocal + broadcast + one
    wide multiply) and the wo projection for those tokens run overlapped
    with the remaining attention.
Output: core c writes out rows [128u + r] = flat token 1024u + 128c + r.
"""

import numpy as np
import ml_dtypes

import concourse.bass as bass
import concourse.mybir as mybir
import concourse.tile as tile
from concourse import bacc
from concourse.bass_utils import run_bass_kernel_spmd

BF16 = ml_dtypes.bfloat16
F32 = np.float32

B, S, HID = 2, 2048, 2048
H, KVH, D = 32, 8, 64
NC = 8
T = B * S              # 4096 flat tokens
LH = H // NC           # 4 local q-heads (2 pairs)
TC = 512               # token chunk
NTC = T // TC          # 8 chunks
KB = 128               # key block
NG = 4                 # a2a token groups (1024 flat tokens each)
GT = T // NG // NC     # 128 tokens per core per group

_CACHE = {}


def _build():
    import os
    DBG = os.environ.get("KDEBUG", "0") == "1"
    fp32 = mybir.dt.float32
    bf16 = mybir.dt.bfloat16

    nc = bacc.Bacc("TRN2", target_bir_lowering=False, debug=False, num_devices=NC)

    # all inputs pre-tiled on host so every DMA is one contiguous run per
    # partition (128 descriptors instead of 2048)
    xT = nc.dram_tensor("xT", [128, NTC, 16, TC], bf16, kind="ExternalInput")
    wq_c = nc.dram_tensor("wq_c", [128, 16, LH * D], bf16, kind="ExternalInput")
    wkv_c = nc.dram_tensor("wkv_c", [128, 16, 2 * D], bf16,
                           kind="ExternalInput")
    wo_d = nc.dram_tensor("wo", [128, 4, 16, 512], bf16, kind="ExternalInput")
    trigq_d = nc.dram_tensor("trigq", [128, NTC, 2, TC], bf16,
                             kind="ExternalInput")
    trigk_d = nc.dram_tensor("trigk", [64, NTC, 2, TC], bf16,
                             kind="ExternalInput")
    maD_d = [nc.dram_tensor(f"maD{j}", [128, 2 * TC], bf16, kind="ExternalInput")
             for j in range(4)]
    out_d = nc.dram_tensor("out", [NG * GT, HID], fp32, kind="ExternalOutput")
    if DBG:
        dbg_qT = nc.dram_tensor("dbg_qT", [2, 128, T], fp32, kind="ExternalOutput")
        dbg_k2 = nc.dram_tensor("dbg_k2", [128, T], fp32, kind="ExternalOutput")
        dbg_attnT = nc.dram_tensor("dbg_attnT", [2, 128, T], fp32,
                                   kind="ExternalOutput")
        dbg_den = nc.dram_tensor("dbg_den", [NG, 2, 2, 1024], fp32,
                                 kind="ExternalOutput")
        dbg_ao = nc.dram_tensor("dbg_ao", [NG, 128, 16, GT], fp32,
                                kind="ExternalOutput")
        dbg_rcpb = nc.dram_tensor("dbg_rcpb", [NG, 128, 16, GT], fp32,
                                  kind="ExternalOutput")
        dbg_denall = nc.dram_tensor("dbg_denall", [NG, 64, GT], fp32,
                                    kind="ExternalOutput")
        dbg_rcp = nc.dram_tensor("dbg_rcp", [NG, 64, GT], fp32,
                                 kind="ExternalOutput")


    with tile.TileContext(nc) as tc:
        with (
            tc.tile_pool(name="persist", bufs=1) as persist,
            tc.tile_pool(name="stream", bufs=2) as stream,
            tc.tile_pool(name="work", bufs=2) as work,
            tc.tile_pool(name="psum", bufs=1, space="PSUM") as psum,
            tc.tile_pool(name="dram", bufs=1, space="DRAM") as dram,
        ):
            # ---- prologue: exp table load warm-up ----
            dummy = work.tile([1, 2], fp32, tag="dummy", name="dummy")
            nc.gpsimd.memset(dummy[:], 0.0)
            nc.scalar.activation(dummy[:], dummy[:],
                                 mybir.ActivationFunctionType.Exp)

            # ---- persistent tiles ----
            qT = [persist.tile([128, T], bf16, tag=f"qT{t}", name=f"qT{t}")
                  for t in range(2)]
            k2 = persist.tile([128, T], bf16, tag="k2", name="k2")
            vatt = [persist.tile([128, D + 1], bf16, tag=f"vatt{i}",
                                 name=f"vatt{i}") for i in range(T // KB)]
            attnT = [persist.tile([128, T], bf16, tag=f"attnT{t}",
                                  name=f"attnT{t}") for t in range(2)]
            den_tiles = {}  # (group u, pair p) -> [1, 2, 1024] bf16 tile
            ident = persist.tile([128, 128], bf16, tag="ident", name="ident")

            # weights: single big DMAs on the ACT HWDGE ring (scalar queue is
            # otherwise idle until the first EXP), so they overlap the x
            # chunk-0 load on the SP ring
            wq_sb = persist.tile([128, 16, LH * D], bf16, tag="wq", name="wq")
            wkv_sb = persist.tile([128, 16, 2 * D], bf16, tag="wkv", name="wkv")
            nc.scalar.dma_start(wq_sb[:], wq_c[:])
            nc.scalar.dma_start(wkv_sb[:], wkv_c[:])

            maD = []
            for j in range(4):
                mt = persist.tile([128, 2 * TC], bf16, tag=f"maD{j}",
                                  name=f"maD{j}")
                nc.gpsimd.dma_start(mt[:], maD_d[j][:])
                maD.append(mt)
            from concourse.masks import make_identity
            make_identity(nc, ident[:])
            for i in range(T // KB):
                nc.gpsimd.memset(vatt[i][:, D:D + 1], 1.0)

            # wo: 4 column blocks, resident
            wo_sb = []
            for nt in range(4):
                t_ = persist.tile([128, 16, 512], bf16, tag=f"wo{nt}",
                                  name=f"wo{nt}")
                wo_sb.append(t_)

            # a2a staging (one group = 1024 flat tokens; slot j -> core j's
            # 128 tokens; rows 0:128 attnT block, 128:130 the 2 denominators).
            # One collective per (group, pair) so pair 0's redistribution
            # overlaps pair 1's attention.
            a2a_in = [[dram.tile([NC, 130, GT], bf16, tag=f"a2a_in{u}_{p}",
                                 name=f"a2a_in{u}_{p}") for p in range(2)]
                      for u in range(NG)]
            a2a_out = [[dram.tile([NC, 130, GT], bf16, tag=f"a2a_out{u}_{p}",
                                  name=f"a2a_out{u}_{p}") for p in range(2)]
                       for u in range(NG)]
            rstage = [dram.tile([2, 2, 8, GT], bf16, tag=f"rstage{u}",
                                name=f"rstage{u}") for u in range(NG)]

            def rope_q(out_ap, ps, tg, tsl):
                ct = tg[:, 0, :]
                st = tg[:, 1, :]
                t1 = work.tile([128, TC], fp32, tag="rope_t1", bufs=1, name="t1")
                t2 = work.tile([128, TC], fp32, tag="rope_t2", bufs=1, name="t2")
                nc.vector.tensor_mul(t1[:], ps[:], ct)
                for base in range(0, 128, 64):
                    a, b = base, base + 32
                    nc.vector.tensor_mul(t2[a:a + 32, :], ps[b:b + 32, :],
                                         st[a:a + 32, :])
                    nc.vector.tensor_mul(t2[b:b + 32, :], ps[a:a + 32, :],
                                         st[b:b + 32, :])
                nc.vector.tensor_add(out_ap, t1[:], t2[:])

            def rope_k(out_ap, ps, tg):
                ct = tg[:, 0, :]
                st = tg[:, 1, :]
                t1 = work.tile([64, TC], fp32, tag="rope_t1", bufs=1, name="kt1")
                t2 = work.tile([64, TC], fp32, tag="rope_t2", bufs=1, name="kt2")
                nc.vector.tensor_mul(t1[:], ps[0:64, :], ct)
                nc.vector.tensor_mul(t2[0:32, :], ps[32:64, :], st[0:32, :])
                nc.vector.tensor_mul(t2[32:64, :], ps[0:32, :], st[32:64, :])
                nc.vector.tensor_add(out_ap, t1[:], t2[:])

            def emit_a2a(u, p):
                """Stage + send pair p's slice of group u."""
                nc.gpsimd.dma_start(
                    a2a_in[u][p][:, 0:128, :].rearrange("j q t -> q j t"),
                    attnT[p][:, 1024 * u:1024 * (u + 1)]
                    .rearrange("q (j t) -> q j t", j=NC))
                for h in range(2):
                    nc.gpsimd.dma_start(
                        a2a_in[u][p][:, 128 + h, :],
                        den_tiles[(u, p)][:, 1024 * h:1024 * (h + 1)]
                        .rearrange("o (j t) -> o j t", j=NC))
                nc.gpsimd.collective_compute(
                    "AllToAll", mybir.AluOpType.bypass,
                    replica_groups=[list(range(NC))],
                    ins=[a2a_in[u][p].opt()], outs=[a2a_out[u][p].opt()])
                if DBG:
                    nc.gpsimd.dma_start(
                        dbg_den[u, p, :, :],
                        den_tiles[(u, p)][:, :]
                        .rearrange("o (h t) -> o h t", h=2))

            p2_state = {}

            def _setup_parity(u, pp):
                """Load + normalize parity-pp slots (kk = 2r + pp) of group u."""
                if u not in p2_state:
                    p2_state[u] = (
                        stream.tile([128, 16, GT], bf16, tag="aoall", bufs=1,
                                    name=f"aoall{u}"),
                        work.tile([64, GT], bf16, tag="den_all",
                                  name=f"den_all{u}"),
                        work.tile([64, GT], bf16, tag="rcp", name=f"rcp{u}"),
                        stream.tile([128, 16, GT], bf16, tag="rcpb", bufs=1,
                                    name=f"rcpb{u}"))
                aoall, den_all, rcp, rcpb = p2_state[u]
                for r in range(NC):
                    eng = nc.sync if r % 2 == 0 else nc.gpsimd
                    eng.dma_start(aoall[:, 2 * r + pp, :],
                                  a2a_out[u][pp][r, 0:128, :])
                # den_all rows: 32*pp + 8*h + r (parity halves 32-aligned for
                # the DVE reciprocal)
                for h in range(2):
                    nc.sync.dma_start(
                        den_all[32 * pp + 8 * h:32 * pp + 8 * h + 8, :],
                        a2a_out[u][pp][:, 128 + h, :])
                with nc.allow_low_precision(reason="bf16 softmax denominators"):
                    nc.vector.reciprocal(rcp[32 * pp:32 * pp + 16, :],
                                         den_all[32 * pp:32 * pp + 16, :])
                nc.sync.dma_start(
                    rstage[u][pp].rearrange("h r t -> (h r) t"),
                    rcp[32 * pp:32 * pp + 16, :])
                rcpb4 = rcpb[:].rearrange("q (r pp) t -> q r pp t", pp=2)
                for h in range(2):
                    nc.gpsimd.dma_start(
                        rcpb4[64 * h:64 * (h + 1), :, pp, :],
                        rstage[u][pp:pp + 1, h, :, :].broadcast_to([64, 8, GT]))
                ao4 = aoall[:].rearrange("q (r pp) t -> q r pp t", pp=2)
                nc.vector.tensor_mul(ao4[:, :, pp, :], ao4[:, :, pp, :],
                                     rcpb4[:, :, pp, :])
                if DBG and pp == 1:
                    nc.gpsimd.dma_start(dbg_rcpb[u, :, :, :], rcpb[:])
                    nc.gpsimd.dma_start(dbg_denall[u, :, :], den_all[:])
                    nc.gpsimd.dma_start(dbg_rcp[u, :, :], rcp[:])
                    nc.gpsimd.dma_start(dbg_ao[u, :, :, :], aoall[:])

            def _p2_mm(u, nt, ps_ap, kks):
                aoall = p2_state[u][0]
                for kk in kks:
                    nc.tensor.matmul(ps_ap, aoall[:, kk, :],
                                     wo_sb[nt][:, kk, :],
                                     start=(kk == 0), stop=(kk == 15))

            def _p2_drain(u, nt, ps_ap):
                ob = work.tile([128, 512], fp32, tag="ob", name=f"ob{u}_{nt}")
                nc.vector.tensor_copy(ob[:], ps_ap)
                nc.gpsimd.dma_start(
                    out_d[GT * u:GT * (u + 1), 512 * nt:512 * (nt + 1)], ob[:])

            def _p3_accums():
                tiles = p2_state["ps3"]
                return [tiles[0][:], tiles[1][:],
                        tiles[2][:, 0:512], tiles[2][:, 512:1024]]

            def emit_phase2(u, half):
                """Consume group u: normalize + project its 128 token rows."""
                if u < 3:
                    if half == 0:
                        _setup_parity(u, 0)
                        _setup_parity(u, 1)
                    for nt in (0, 1) if half == 0 else (2, 3):
                        ps = psum.tile([128, 512], fp32, tag="mm", bufs=2,
                                       name=f"ps_o{u}_{nt}")
                        _p2_mm(u, nt, ps[:], range(16))
                        _p2_drain(u, nt, ps[:])
                    return
                # u == 3 (tail): evens (pair-0 data) first across all 4 nt so
                # the PE starts before pair 1's collective lands.
                if half == 0:
                    _setup_parity(u, 0)
                    p2_state["ps3"] = [
                        psum.tile([128, 512], fp32, tag="mm", bufs=2,
                                  name="ps_o3_0"),
                        psum.tile([128, 512], fp32, tag="mm", bufs=2,
                                  name="ps_o3_1"),
                        psum.tile([128, 1024], fp32, tag="psO", bufs=1,
                                  name="ps_o3_23")]
                    for nt, ps_ap in enumerate(_p3_accums()):
                        _p2_mm(u, nt, ps_ap, range(0, 16, 2))
                else:
                    _setup_parity(u, 1)
                    for nt, ps_ap in enumerate(_p3_accums()):
                        _p2_mm(u, nt, ps_ap, range(1, 16, 2))
                        _p2_drain(u, nt, ps_ap)

            # ================= main interleaved loop =================
            for t8 in range(NTC):
                b, cq = t8 // 4, t8 % 4
                tsl = slice(TC * t8, TC * (t8 + 1))

                xq = stream.tile([128, 16, TC], bf16, tag="xq", name=f"xq{t8}")
                nc.sync.dma_start(xq[:], xT[:, t8, :, :])
                tgq = stream.tile([128, 2, TC], bf16, tag="tgq", bufs=1,
                                  name=f"tgq{t8}")
                tgk = stream.tile([64, 2, TC], bf16, tag="tgk", bufs=1,
                                  name=f"tgk{t8}")
                nc.sync.dma_start(tgq[:], trigq_d[:, t8, :, :])
                nc.sync.dma_start(tgk[:], trigk_d[:, t8, :, :])
                if t8 < 4:
                    nc.sync.dma_start(wo_sb[t8][:], wo_d[:, t8, :, :])

                # ---- QKV projections + RoPE ----
                for qt in range(2):
                    ps = psum.tile([128, TC], fp32, tag="mm", bufs=2,
                                   name=f"ps_q{t8}_{qt}")
                    for k in range(16):
                        nc.tensor.matmul(ps[:],
                                         wq_sb[:, k, 128 * qt:128 * (qt + 1)],
                                         xq[:, k, :],
                                         start=(k == 0), stop=(k == 15))
                    rope_q(qT[qt][:, tsl], ps, tgq, tsl)

                ps = psum.tile([128, TC], fp32, tag="mm", bufs=2,
                               name=f"ps_kv{t8}")
                for k in range(16):
                    nc.tensor.matmul(ps[:], wkv_sb[:, k, :], xq[:, k, :],
                                     start=(k == 0), stop=(k == 15))
                rope_k(k2[0:64, tsl], ps, tgk)
                nc.vector.tensor_copy(k2[64:128, tsl], k2[0:64, tsl])
                vt = work.tile([64, TC], bf16, tag="vt", name=f"vt{t8}")
                nc.vector.tensor_copy(vt[:], ps[64:128, :])
                for j in range(TC // KB):
                    kbi = (TC // KB) * t8 + j
                    pst = psum.tile([128, TC], bf16, tag="mm", bufs=2,
                                    name=f"ps_tr{t8}_{j}")
                    nc.tensor.transpose(pst[:, 0:64],
                                        vt[:, 128 * j:128 * (j + 1)],
                                        ident[0:64, 0:64])
                    nc.scalar.copy(vatt[kbi][:, 0:D], pst[:, 0:64])

                # ---- attention for (b, cq), both pairs ----
                nkb = 4 * (cq + 1)
                qs = S * b + TC * cq
                gu, goff = qs // 1024, qs % 1024
                for p in range(2):
                    if (gu, p) not in den_tiles:
                        den_tiles[(gu, p)] = work.tile(
                            [1, 2048], bf16, tag=f"deng{p}", bufs=2,
                            name=f"deng{gu}_{p}")
                    qtile = qT[p]
                    psO = psum.tile([D + 1, 1024], fp32, tag="psO", bufs=1,
                                    name=f"psO{t8}_{p}")

                    def emit_av(kb, ex):
                        vt_ = vatt[(S // KB) * b + kb][:]
                        nc.tensor.matmul(psO[:, 0:512], vt_, ex[:, 0:512],
                                         start=(kb == 0), stop=(kb == nkb - 1))
                        nc.tensor.matmul(psO[:, 512:1024], vt_,
                                         ex[:, 512:1024],
                                         start=(kb == 0), stop=(kb == nkb - 1))

                    # 2-kblock skew: S^T(kb) runs while AV(kb-2) waits on
                    # exp/psO, so the in-order PE queue never starves.
                    pend = []
                    for kb in range(nkb):
                        kpos = S * b + KB * kb
                        psS = psum.tile([128, 1024], fp32, tag="psS", bufs=2,
                                        name=f"psS{t8}_{p}_{kb}")
                        nc.tensor.matmul(psS[:, 0:512],
                                         k2[0:64, kpos:kpos + KB],
                                         qtile[0:64, qs:qs + TC],
                                         start=True, stop=True)
                        nc.tensor.matmul(psS[:, 512:1024],
                                         k2[64:128, kpos:kpos + KB],
                                         qtile[64:128, qs:qs + TC],
                                         start=True, stop=True)
                        ex = work.tile([128, 1024], bf16, tag="ex", bufs=3,
                                       name=f"ex{t8}_{p}_{kb}")
                        nc.scalar.activation(ex[:], psS[:],
                                             mybir.ActivationFunctionType.Exp)
                        if kb >= nkb - 4:
                            nc.vector.tensor_mul(ex[:], ex[:],
                                                 maD[kb - (nkb - 4)][:])
                        pend.append((kb, ex))
                        if len(pend) > 2:
                            emit_av(*pend.pop(0))
                    for it in pend:
                        emit_av(*it)
                    # drain
                    nc.vector.tensor_copy(attnT[p][0:64, qs:qs + TC],
                                          psO[0:64, 0:512])
                    nc.vector.tensor_copy(attnT[p][64:128, qs:qs + TC],
                                          psO[0:64, 512:1024])
                    for h in range(2):
                        nc.vector.tensor_copy(
                            den_tiles[(gu, p)][:, 1024 * h + goff:
                                               1024 * h + goff + TC],
                            psO[64:65, 512 * h:512 * (h + 1)])
                    if t8 % 2 == 1:
                        emit_a2a(t8 // 2, p)
                    if t8 >= 3 and t8 % 2 == 1:
                        emit_phase2((t8 - 3) // 2, p)
            emit_phase2(3, 0)
            emit_phase2(3, 1)
            if DBG:
                for p in range(2):
                    nc.gpsimd.dma_start(dbg_qT[p, :, :], qT[p][:])
                    nc.gpsimd.dma_start(dbg_attnT[p, :, :], attnT[p][:])
                nc.gpsimd.dma_start(dbg_k2[:, :], k2[:])

    nc.compile()
    return nc


def _prep_inputs(x, cos, sin, wq, wk, wv, wo):
    x = np.asarray(x, F32)
    cos = np.asarray(cos, F32)
    sin = np.asarray(sin, F32)
    wq = np.asarray(wq, F32)
    wk = np.asarray(wk, F32)
    wv = np.asarray(wv, F32)
    wo = np.asarray(wo, F32)

    # [128, chunk, k, t] = x[512*chunk + t, 128*k + p]
    xT = np.ascontiguousarray(
        x.reshape(NTC, TC, 16, 128).transpose(3, 0, 2, 1)).astype(BF16)
    # [128, nt, k, m] = wo[128*k + p, 512*nt + m]
    wo_b = np.ascontiguousarray(
        wo.reshape(16, 128, 4, 512).transpose(1, 2, 0, 3)).astype(BF16)

    pos = np.arange(T) % S
    sign = np.concatenate([-np.ones(D // 2, F32), np.ones(D // 2, F32)])
    ctk = np.ascontiguousarray(cos[pos].T)                      # [64, T]
    stk = np.ascontiguousarray((sin[pos] * sign).T)             # [64, T]
    sc = F32(1.0 / np.sqrt(D))
    ctq = np.concatenate([ctk, ctk], 0) * sc                    # [128, T]
    stq = np.concatenate([stk, stk], 0) * sc

    def chunked(ct, st):  # [P, T] x2 -> [P, NTC, 2, TC]
        P = ct.shape[0]
        s = np.stack([ct, st], axis=1)          # [P, 2, T]
        return np.ascontiguousarray(
            s.reshape(P, 2, NTC, TC).transpose(0, 2, 1, 3)).astype(BF16)

    trigq = chunked(ctq, stq)
    trigk = chunked(ctk, stk)

    ql = np.arange(TC)
    kl = np.arange(128)
    maD = []
    for j in range(4):
        m = (ql[None, :] >= (kl[:, None] + 128 * j)).astype(BF16)
        maD.append(np.concatenate([m, m], axis=1))              # [128, 1024]

    in_maps = []
    for c in range(NC):
        wq_cc = np.ascontiguousarray(
            wq[:, c * LH * D:(c + 1) * LH * D]
            .reshape(16, 128, LH * D).transpose(1, 0, 2)).astype(BF16)
        wkv_cc = np.ascontiguousarray(np.concatenate(
            [wk[:, c * D:(c + 1) * D], wv[:, c * D:(c + 1) * D]], 1)
            .reshape(16, 128, 2 * D).transpose(1, 0, 2)).astype(BF16)
        in_maps.append({
            "xT": xT, "wq_c": wq_cc, "wkv_c": wkv_cc, "wo": wo_b,
            "trigq": trigq, "trigk": trigk,
            "maD0": maD[0], "maD1": maD[1], "maD2": maD[2], "maD3": maD[3],
        })
    return in_maps


def get_nc():
    if "nc" not in _CACHE:
        _CACHE["nc"] = _build()
    return _CACHE["nc"]


def run(in_maps, **kwargs):
    nc = get_nc()
    return run_bass_kernel_spmd(nc, in_maps, core_ids=list(range(NC)), **kwargs)


def kernel(x, cos, sin, wq, wk, wv, wo):
    in_maps = _prep_inputs(x, cos, sin, wq, wk, wv, wo)
    res = run(in_maps)
    out = np.empty((T, HID), F32)
    for c in range(NC):
        r = res.results[c]["out"]
        for u in range(NG):
            out[1024 * u + GT * c:1024 * u + GT * (c + 1)] = \
                r[GT * u:GT * (u + 1)]
    return out.reshape(B, S, HID)


# revision 85
# speedup vs baseline: 1.0372x; 1.0207x over previous
"""Distributed Bass kernel for nn_Attention (B=2, S=2048, HID=2048, H=32, KVH=8, D=64).

Sharding (8 cores, uniform SPMD): core c owns kv-head c and its 4 GQA query
heads (2 pairs).  x replicated as xT [HID, T] bf16.

Pipeline (emitted interleaved so all engines overlap):
  per 512-token chunk t (b = t//4, cq = t%4):
    - one 2MB DMA for the x chunk, one DMA per trig table chunk
    - QKV projections (PE) + RoPE (DVE) -> qT/k2/vatt
    - causal attention for (b, cq), both head pairs: per 128-key block one
      psS [128, 2*512] holds both heads' scores (S^T matmuls on disjoint PE
      row groups run concurrently); one EXP (ACT) covers both; AV shares one
      LDWEIGHTS.  Denominator comes from a ones-column in vatt (psO row 64).
  - after every 2 chunks (1024 tokens), an AllToAll redistributes that token
    group's unnormalized attn^T (+2 denominator rows in-band) so every core
    gets its 128-token slice; normalization (reciprocal + broadcast + one
    wide multiply) and the wo projection for those tokens run overlapped
    with the remaining attention.
Output: core c writes out rows [128u + r] = flat token 1024u + 128c + r.
"""

import numpy as np
import ml_dtypes

import concourse.bass as bass
import concourse.mybir as mybir
import concourse.tile as tile
from concourse import bacc
from concourse.bass_utils import run_bass_kernel_spmd

BF16 = ml_dtypes.bfloat16
F32 = np.float32

B, S, HID = 2, 2048, 2048
H, KVH, D = 32, 8, 64
NC = 8
T = B * S              # 4096 flat tokens
LH = H // NC           # 4 local q-heads (2 pairs)
TC = 512               # token chunk
NTC = T // TC          # 8 chunks
KB = 128               # key block
NG = 4                 # a2a token groups (1024 flat tokens each)
GT = T // NG // NC     # 128 tokens per core per group

_CACHE = {}


def _build():
    import os
    DBG = os.environ.get("KDEBUG", "0") == "1"
    fp32 = mybir.dt.float32
    bf16 = mybir.dt.bfloat16

    nc = bacc.Bacc("TRN2", target_bir_lowering=False, debug=False, num_devices=NC)

    # all inputs pre-tiled on host so every DMA is one contiguous run per
    # partition (128 descriptors instead of 2048)
    xT = nc.dram_tensor("xT", [128, NTC, 16, TC], bf16, kind="ExternalInput")
    wq_c = nc.dram_tensor("wq_c", [128, 16, LH * D], bf16, kind="ExternalInput")
    wkv_c = nc.dram_tensor("wkv_c", [128, 16, 2 * D], bf16,
                           kind="ExternalInput")
    wo_d = nc.dram_tensor("wo", [128, 4, 16, 512], bf16, kind="ExternalInput")
    trigq_d = nc.dram_tensor("trigq", [128, NTC, 2, TC], bf16,
                             kind="ExternalInput")
    trigk_d = nc.dram_tensor("trigk", [64, NTC, 2, TC], bf16,
                             kind="ExternalInput")
    maD_d = [nc.dram_tensor(f"maD{j}", [128, 2 * TC], bf16, kind="ExternalInput")
             for j in range(4)]
    out_d = nc.dram_tensor("out", [NG * GT, HID], fp32, kind="ExternalOutput")
    if DBG:
        dbg_qT = nc.dram_tensor("dbg_qT", [2, 128, T], fp32, kind="ExternalOutput")
        dbg_k2 = nc.dram_tensor("dbg_k2", [128, T], fp32, kind="ExternalOutput")
        dbg_attnT = nc.dram_tensor("dbg_attnT", [2, 128, T], fp32,
                                   kind="ExternalOutput")
        dbg_den = nc.dram_tensor("dbg_den", [NG, 2, 2, 1024], fp32,
                                 kind="ExternalOutput")
        dbg_ao = nc.dram_tensor("dbg_ao", [NG, 128, 16, GT], fp32,
                                kind="ExternalOutput")
        dbg_rcpb = nc.dram_tensor("dbg_rcpb", [NG, 128, 16, GT], fp32,
                                  kind="ExternalOutput")
        dbg_denall = nc.dram_tensor("dbg_denall", [NG, 64, GT], fp32,
                                    kind="ExternalOutput")
        dbg_rcp = nc.dram_tensor("dbg_rcp", [NG, 64, GT], fp32,
                                 kind="ExternalOutput")


    with tile.TileContext(nc) as tc:
        with (
            tc.tile_pool(name="persist", bufs=1) as persist,
            tc.tile_pool(name="stream", bufs=2) as stream,
            tc.tile_pool(name="work", bufs=2) as work,
            tc.tile_pool(name="psum", bufs=1, space="PSUM") as psum,
            tc.tile_pool(name="dram", bufs=1, space="DRAM") as dram,
        ):
            # ---- prologue: exp table load warm-up ----
            dummy = work.tile([1, 2], fp32, tag="dummy", name="dummy")
            nc.gpsimd.memset(dummy[:], 0.0)
            nc.scalar.activation(dummy[:], dummy[:],
                                 mybir.ActivationFunctionType.Exp)

            # ---- persistent tiles ----
            qT = [persist.tile([128, T], bf16, tag=f"qT{t}", name=f"qT{t}")
                  for t in range(2)]
            k2 = persist.tile([128, T], bf16, tag="k2", name="k2")
            vatt = [persist.tile([128, D + 1], bf16, tag=f"vatt{i}",
                                 name=f"vatt{i}") for i in range(T // KB)]
            attnT = [persist.tile([128, T], bf16, tag=f"attnT{t}",
                                  name=f"attnT{t}") for t in range(2)]
            den_tiles = {}  # (group u, pair p) -> [1, 2, 1024] bf16 tile
            ident = persist.tile([128, 128], bf16, tag="ident", name="ident")

            # weights: single big DMAs on the ACT HWDGE ring (scalar queue is
            # otherwise idle until the first EXP), so they overlap the x
            # chunk-0 load on the SP ring
            wq_sb = persist.tile([128, 16, LH * D], bf16, tag="wq", name="wq")
            wkv_sb = persist.tile([128, 16, 2 * D], bf16, tag="wkv", name="wkv")
            nc.scalar.dma_start(wq_sb[:], wq_c[:])
            nc.scalar.dma_start(wkv_sb[:], wkv_c[:])

            maD = []
            for j in range(4):
                mt = persist.tile([128, 2 * TC], bf16, tag=f"maD{j}",
                                  name=f"maD{j}")
                nc.gpsimd.dma_start(mt[:], maD_d[j][:])
                maD.append(mt)
            from concourse.masks import make_identity
            make_identity(nc, ident[:])
            for i in range(T // KB):
                nc.gpsimd.memset(vatt[i][:, D:D + 1], 1.0)

            # wo: 4 column blocks, resident
            wo_sb = []
            for nt in range(4):
                t_ = persist.tile([128, 16, 512], bf16, tag=f"wo{nt}",
                                  name=f"wo{nt}")
                wo_sb.append(t_)

            # a2a staging (one group = 1024 flat tokens; slot j -> core j's
            # 128 tokens; rows 0:128 attnT block, 128:130 the 2 denominators).
            # One collective per (group, pair) so pair 0's redistribution
            # overlaps pair 1's attention.
            a2a_in = [[dram.tile([NC, 130, GT], bf16, tag=f"a2a_in{u}_{p}",
                                 name=f"a2a_in{u}_{p}") for p in range(2)]
                      for u in range(NG)]
            a2a_out = [[dram.tile([NC, 130, GT], bf16, tag=f"a2a_out{u}_{p}",
                                  name=f"a2a_out{u}_{p}") for p in range(2)]
                       for u in range(NG)]
            rstage = [dram.tile([2, 2, 8, GT], bf16, tag=f"rstage{u}",
                                name=f"rstage{u}") for u in range(NG)]

            def rope_q(out_ap, ps, tg, tsl):
                ct = tg[:, 0, :]
                st = tg[:, 1, :]
                t1 = work.tile([128, TC], fp32, tag="rope_t1", bufs=1, name="t1")
                t2 = work.tile([128, TC], fp32, tag="rope_t2", bufs=1, name="t2")
                nc.vector.tensor_mul(t1[:], ps[:], ct)
                for base in range(0, 128, 64):
                    a, b = base, base + 32
                    nc.vector.tensor_mul(t2[a:a + 32, :], ps[b:b + 32, :],
                                         st[a:a + 32, :])
                    nc.vector.tensor_mul(t2[b:b + 32, :], ps[a:a + 32, :],
                                         st[b:b + 32, :])
                nc.vector.tensor_add(out_ap, t1[:], t2[:])

            def rope_k(out_ap, ps, tg):
                ct = tg[:, 0, :]
                st = tg[:, 1, :]
                t1 = work.tile([64, TC], fp32, tag="rope_t1", bufs=1, name="kt1")
                t2 = work.tile([64, TC], fp32, tag="rope_t2", bufs=1, name="kt2")
                nc.vector.tensor_mul(t1[:], ps[0:64, :], ct)
                nc.vector.tensor_mul(t2[0:32, :], ps[32:64, :], st[0:32, :])
                nc.vector.tensor_mul(t2[32:64, :], ps[0:32, :], st[32:64, :])
                nc.vector.tensor_add(out_ap, t1[:], t2[:])

            def emit_a2a(u, p):
                """Stage + send pair p's slice of group u."""
                nc.gpsimd.dma_start(
                    a2a_in[u][p][:, 0:128, :].rearrange("j q t -> q j t"),
                    attnT[p][:, 1024 * u:1024 * (u + 1)]
                    .rearrange("q (j t) -> q j t", j=NC))
                for h in range(2):
                    nc.gpsimd.dma_start(
                        a2a_in[u][p][:, 128 + h, :],
                        den_tiles[(u, p)][:, 1024 * h:1024 * (h + 1)]
                        .rearrange("o (j t) -> o j t", j=NC))
                nc.gpsimd.collective_compute(
                    "AllToAll", mybir.AluOpType.bypass,
                    replica_groups=[list(range(NC))],
                    ins=[a2a_in[u][p].opt()], outs=[a2a_out[u][p].opt()])
                if DBG:
                    nc.gpsimd.dma_start(
                        dbg_den[u, p, :, :],
                        den_tiles[(u, p)][:, :]
                        .rearrange("o (h t) -> o h t", h=2))

            p2_state = {}

            def _setup_parity(u, pp):
                """Load + normalize parity-pp slots (kk = 2r + pp) of group u."""
                if u not in p2_state:
                    p2_state[u] = (
                        stream.tile([128, 16, GT], bf16, tag="aoall", bufs=1,
                                    name=f"aoall{u}"),
                        work.tile([64, GT], bf16, tag="den_all",
                                  name=f"den_all{u}"),
                        work.tile([64, GT], bf16, tag="rcp", name=f"rcp{u}"),
                        stream.tile([128, 16, GT], bf16, tag="rcpb", bufs=1,
                                    name=f"rcpb{u}"))
                aoall, den_all, rcp, rcpb = p2_state[u]
                for r in range(NC):
                    eng = nc.sync if r % 2 == 0 else nc.gpsimd
                    eng.dma_start(aoall[:, 2 * r + pp, :],
                                  a2a_out[u][pp][r, 0:128, :])
                # den_all rows: 32*pp + 8*h + r (parity halves 32-aligned for
                # the DVE reciprocal)
                for h in range(2):
                    nc.sync.dma_start(
                        den_all[32 * pp + 8 * h:32 * pp + 8 * h + 8, :],
                        a2a_out[u][pp][:, 128 + h, :])
                with nc.allow_low_precision(reason="bf16 softmax denominators"):
                    nc.vector.reciprocal(rcp[32 * pp:32 * pp + 16, :],
                                         den_all[32 * pp:32 * pp + 16, :])
                nc.sync.dma_start(
                    rstage[u][pp].rearrange("h r t -> (h r) t"),
                    rcp[32 * pp:32 * pp + 16, :])
                rcpb4 = rcpb[:].rearrange("q (r pp) t -> q r pp t", pp=2)
                for h in range(2):
                    nc.gpsimd.dma_start(
                        rcpb4[64 * h:64 * (h + 1), :, pp, :],
                        rstage[u][pp:pp + 1, h, :, :].broadcast_to([64, 8, GT]))
                ao4 = aoall[:].rearrange("q (r pp) t -> q r pp t", pp=2)
                nc.vector.tensor_mul(ao4[:, :, pp, :], ao4[:, :, pp, :],
                                     rcpb4[:, :, pp, :])
                if DBG and pp == 1:
                    nc.gpsimd.dma_start(dbg_rcpb[u, :, :, :], rcpb[:])
                    nc.gpsimd.dma_start(dbg_denall[u, :, :], den_all[:])
                    nc.gpsimd.dma_start(dbg_rcp[u, :, :], rcp[:])
                    nc.gpsimd.dma_start(dbg_ao[u, :, :, :], aoall[:])

            def _p2_mm(u, nt, ps_ap, kks):
                aoall = p2_state[u][0]
                for kk in kks:
                    nc.tensor.matmul(ps_ap, aoall[:, kk, :],
                                     wo_sb[nt][:, kk, :],
                                     start=(kk == 0), stop=(kk == 15))

            def _p2_drain(u, nt, ps_ap):
                ob = work.tile([128, 512], fp32, tag="ob", name=f"ob{u}_{nt}")
                nc.vector.tensor_copy(ob[:], ps_ap)
                nc.gpsimd.dma_start(
                    out_d[GT * u:GT * (u + 1), 512 * nt:512 * (nt + 1)], ob[:])

            def _p3_accums():
                tiles = p2_state["ps3"]
                return [tiles[0][:], tiles[1][:],
                        tiles[2][:, 0:512], tiles[2][:, 512:1024]]

            def emit_phase2(u, half):
                """Consume group u: normalize + project its 128 token rows."""
                if u < 3:
                    if half == 0:
                        _setup_parity(u, 0)
                        _setup_parity(u, 1)
                    for nt in (0, 1) if half == 0 else (2, 3):
                        ps = psum.tile([128, 512], fp32, tag="mm", bufs=2,
                                       name=f"ps_o{u}_{nt}")
                        _p2_mm(u, nt, ps[:], range(16))
                        _p2_drain(u, nt, ps[:])
                    return
                # u == 3 (tail): evens (pair-0 data) first across all 4 nt so
                # the PE starts before pair 1's collective lands.
                if half == 0:
                    _setup_parity(u, 0)
                    p2_state["ps3"] = [
                        psum.tile([128, 512], fp32, tag="mm", bufs=2,
                                  name="ps_o3_0"),
                        psum.tile([128, 512], fp32, tag="mm", bufs=2,
                                  name="ps_o3_1"),
                        psum.tile([128, 1024], fp32, tag="psO", bufs=1,
                                  name="ps_o3_23")]
                    for nt, ps_ap in enumerate(_p3_accums()):
                        _p2_mm(u, nt, ps_ap, range(0, 16, 2))
                else:
                    _setup_parity(u, 1)
                    for nt, ps_ap in enumerate(_p3_accums()):
                        _p2_mm(u, nt, ps_ap, range(1, 16, 2))
                        _p2_drain(u, nt, ps_ap)

            # ================= main interleaved loop =================
            for t8 in range(NTC):
                b, cq = t8 // 4, t8 % 4
                tsl = slice(TC * t8, TC * (t8 + 1))

                xq = stream.tile([128, 16, TC], bf16, tag="xq", name=f"xq{t8}")
                nc.sync.dma_start(xq[:], xT[:, t8, :, :])
                tgq = stream.tile([128, 2, TC], bf16, tag="tgq", bufs=1,
                                  name=f"tgq{t8}")
                tgk = stream.tile([64, 2, TC], bf16, tag="tgk", bufs=1,
                                  name=f"tgk{t8}")
                nc.sync.dma_start(tgq[:], trigq_d[:, t8, :, :])
                nc.sync.dma_start(tgk[:], trigk_d[:, t8, :, :])
                if t8 < 4:
                    nc.sync.dma_start(wo_sb[t8][:], wo_d[:, t8, :, :])

                # ---- QKV projections + RoPE ----
                # On cq==0 chunks the first attention unit consumes THIS
                # chunk's K, so emit KV first and let the Q matmuls hide the
                # K-rope latency; otherwise Q first (S^T kb0 needs only qT).
                def emit_q():
                    for qt in range(2):
                        ps = psum.tile([128, TC], fp32, tag="mm", bufs=2,
                                       name=f"ps_q{t8}_{qt}")
                        for k in range(16):
                            nc.tensor.matmul(
                                ps[:], wq_sb[:, k, 128 * qt:128 * (qt + 1)],
                                xq[:, k, :], start=(k == 0), stop=(k == 15))
                        rope_q(qT[qt][:, tsl], ps, tgq, tsl)

                def emit_kv():
                    ps = psum.tile([128, TC], fp32, tag="mm", bufs=2,
                                   name=f"ps_kv{t8}")
                    for k in range(16):
                        nc.tensor.matmul(ps[:], wkv_sb[:, k, :], xq[:, k, :],
                                         start=(k == 0), stop=(k == 15))
                    rope_k(k2[0:64, tsl], ps, tgk)
                    nc.vector.tensor_copy(k2[64:128, tsl], k2[0:64, tsl])
                    vt = work.tile([64, TC], bf16, tag="vt", name=f"vt{t8}")
                    nc.vector.tensor_copy(vt[:], ps[64:128, :])
                    for j in range(TC // KB):
                        kbi = (TC // KB) * t8 + j
                        pst = psum.tile([128, TC], bf16, tag="mm", bufs=2,
                                        name=f"ps_tr{t8}_{j}")
                        nc.tensor.transpose(pst[:, 0:64],
                                            vt[:, 128 * j:128 * (j + 1)],
                                            ident[0:64, 0:64])
                        nc.scalar.copy(vatt[kbi][:, 0:D], pst[:, 0:64])

                if cq == 0:
                    emit_kv()
                    emit_q()
                else:
                    emit_q()
                    emit_kv()

                # ---- attention for (b, cq), both pairs ----
                nkb = 4 * (cq + 1)
                qs = S * b + TC * cq
                gu, goff = qs // 1024, qs % 1024
                for p in range(2):
                    if (gu, p) not in den_tiles:
                        den_tiles[(gu, p)] = work.tile(
                            [1, 2048], bf16, tag=f"deng{p}", bufs=2,
                            name=f"deng{gu}_{p}")
                    qtile = qT[p]
                    psO = psum.tile([D + 1, 1024], fp32, tag="psO", bufs=1,
                                    name=f"psO{t8}_{p}")

                    def emit_av(kb, ex):
                        vt_ = vatt[(S // KB) * b + kb][:]
                        nc.tensor.matmul(psO[:, 0:512], vt_, ex[:, 0:512],
                                         start=(kb == 0), stop=(kb == nkb - 1))
                        nc.tensor.matmul(psO[:, 512:1024], vt_,
                                         ex[:, 512:1024],
                                         start=(kb == 0), stop=(kb == nkb - 1))

                    # 2-kblock skew: S^T(kb) runs while AV(kb-2) waits on
                    # exp/psO, so the in-order PE queue never starves.
                    pend = []
                    for kb in range(nkb):
                        kpos = S * b + KB * kb
                        psS = psum.tile([128, 1024], fp32, tag="psS", bufs=2,
                                        name=f"psS{t8}_{p}_{kb}")
                        nc.tensor.matmul(psS[:, 0:512],
                                         k2[0:64, kpos:kpos + KB],
                                         qtile[0:64, qs:qs + TC],
                                         start=True, stop=True)
                        nc.tensor.matmul(psS[:, 512:1024],
                                         k2[64:128, kpos:kpos + KB],
                                         qtile[64:128, qs:qs + TC],
                                         start=True, stop=True)
                        ex = work.tile([128, 1024], bf16, tag="ex", bufs=3,
                                       name=f"ex{t8}_{p}_{kb}")
                        nc.scalar.activation(ex[:], psS[:],
                                             mybir.ActivationFunctionType.Exp)
                        if kb >= nkb - 4:
                            nc.vector.tensor_mul(ex[:], ex[:],
                                                 maD[kb - (nkb - 4)][:])
                        pend.append((kb, ex))
                        if len(pend) > 2:
                            emit_av(*pend.pop(0))
                    for it in pend:
                        emit_av(*it)
                    # drain
                    nc.vector.tensor_copy(attnT[p][0:64, qs:qs + TC],
                                          psO[0:64, 0:512])
                    nc.vector.tensor_copy(attnT[p][64:128, qs:qs + TC],
                                          psO[0:64, 512:1024])
                    for h in range(2):
                        nc.vector.tensor_copy(
                            den_tiles[(gu, p)][:, 1024 * h + goff:
                                               1024 * h + goff + TC],
                            psO[64:65, 512 * h:512 * (h + 1)])
                    if t8 % 2 == 1:
                        emit_a2a(t8 // 2, p)
                    if t8 >= 3 and t8 % 2 == 1:
                        emit_phase2((t8 - 3) // 2, p)
            emit_phase2(3, 0)
            emit_phase2(3, 1)
            if DBG:
                for p in range(2):
                    nc.gpsimd.dma_start(dbg_qT[p, :, :], qT[p][:])
                    nc.gpsimd.dma_start(dbg_attnT[p, :, :], attnT[p][:])
                nc.gpsimd.dma_start(dbg_k2[:, :], k2[:])

    nc.compile()
    return nc


def _prep_inputs(x, cos, sin, wq, wk, wv, wo):
    x = np.asarray(x, F32)
    cos = np.asarray(cos, F32)
    sin = np.asarray(sin, F32)
    wq = np.asarray(wq, F32)
    wk = np.asarray(wk, F32)
    wv = np.asarray(wv, F32)
    wo = np.asarray(wo, F32)

    # [128, chunk, k, t] = x[512*chunk + t, 128*k + p]
    xT = np.ascontiguousarray(
        x.reshape(NTC, TC, 16, 128).transpose(3, 0, 2, 1)).astype(BF16)
    # [128, nt, k, m] = wo[128*k + p, 512*nt + m]
    wo_b = np.ascontiguousarray(
        wo.reshape(16, 128, 4, 512).transpose(1, 2, 0, 3)).astype(BF16)

    pos = np.arange(T) % S
    sign = np.concatenate([-np.ones(D // 2, F32), np.ones(D // 2, F32)])
    ctk = np.ascontiguousarray(cos[pos].T)                      # [64, T]
    stk = np.ascontiguousarray((sin[pos] * sign).T)             # [64, T]
    sc = F32(1.0 / np.sqrt(D))
    ctq = np.concatenate([ctk, ctk], 0) * sc                    # [128, T]
    stq = np.concatenate([stk, stk], 0) * sc

    def chunked(ct, st):  # [P, T] x2 -> [P, NTC, 2, TC]
        P = ct.shape[0]
        s = np.stack([ct, st], axis=1)          # [P, 2, T]
        return np.ascontiguousarray(
            s.reshape(P, 2, NTC, TC).transpose(0, 2, 1, 3)).astype(BF16)

    trigq = chunked(ctq, stq)
    trigk = chunked(ctk, stk)

    ql = np.arange(TC)
    kl = np.arange(128)
    maD = []
    for j in range(4):
        m = (ql[None, :] >= (kl[:, None] + 128 * j)).astype(BF16)
        maD.append(np.concatenate([m, m], axis=1))              # [128, 1024]

    in_maps = []
    for c in range(NC):
        wq_cc = np.ascontiguousarray(
            wq[:, c * LH * D:(c + 1) * LH * D]
            .reshape(16, 128, LH * D).transpose(1, 0, 2)).astype(BF16)
        wkv_cc = np.ascontiguousarray(np.concatenate(
            [wk[:, c * D:(c + 1) * D], wv[:, c * D:(c + 1) * D]], 1)
            .reshape(16, 128, 2 * D).transpose(1, 0, 2)).astype(BF16)
        in_maps.append({
            "xT": xT, "wq_c": wq_cc, "wkv_c": wkv_cc, "wo": wo_b,
            "trigq": trigq, "trigk": trigk,
            "maD0": maD[0], "maD1": maD[1], "maD2": maD[2], "maD3": maD[3],
        })
    return in_maps


def get_nc():
    if "nc" not in _CACHE:
        _CACHE["nc"] = _build()
    return _CACHE["nc"]


def run(in_maps, **kwargs):
    nc = get_nc()
    return run_bass_kernel_spmd(nc, in_maps, core_ids=list(range(NC)), **kwargs)


def kernel(x, cos, sin, wq, wk, wv, wo):
    in_maps = _prep_inputs(x, cos, sin, wq, wk, wv, wo)
    res = run(in_maps)
    out = np.empty((T, HID), F32)
    for c in range(NC):
        r = res.results[c]["out"]
        for u in range(NG):
            out[1024 * u + GT * c:1024 * u + GT * (c + 1)] = \
                r[GT * u:GT * (u + 1)]
    return out.reshape(B, S, HID)
